# revision 6
# baseline (speedup 1.0000x reference)
"""Distributed kNN retrieval (EpisodicSDM) for 8 trn2 cores.

Dispatch A (keys sharded 12800/core, all 2048 queries resident):
  prep: per 512-key batch, norms (square+reduce), grouped rsqrt, Pool
  diag build, 8 f32 diag-matmuls (scale+transpose+quantize to fp8 kT),
  fused ACT psum drain.  Key-prep groups are software-pipelined and
  interleaved with qtile-0 of the main loop.
  main: per 128-query tile, 6 "bigpa" blocks of 2048 keys: 4 fp8
  DoubleRow matmuls -> PSUM [128,2048]; ONE fused ACT copy -> f16 planes;
  ONE DVE tensor-tensor max -> packed (f16 score | slot id) into an
  iota-prefilled i32 buffer; plus a 512-key remainder.  Selection is a
  single MAX8 over all 6400 packed slots (a true top-8: no fold-tree
  positional collisions, which were the old accuracy limiter).
  Slot s < 6144: member rows 2s-(s&511) + {0,512}; else 12288+(s-6144)
  + {0,256}.

Dispatch B (queries sharded 256/core): top-24 of the 64 gathered
candidates, ONE indirect gather per slot fetching BOTH member rows from
a host-permuted slot-pair-major table (halves SWDGE descriptor work),
exact fp32 rescore, top-8 + softmax, value gather, weighted sum.
"""

import sys

_TRN_REPO = "/opt/trn_rl_repo"
if _TRN_REPO not in sys.path:
    sys.path.insert(0, _TRN_REPO)

import concourse.bass as bass
import concourse.mybir as mybir
import concourse.tile as tile
from concourse import bacc
from concourse.masks import make_identity

F32 = mybir.dt.float32
F16 = mybir.dt.float16
I16 = mybir.dt.int16
FP8 = mybir.dt.float8e4
I32 = mybir.dt.int32
ALU = mybir.AluOpType
ACTF = mybir.ActivationFunctionType
AX = mybir.AxisListType

B = 2048
D = 256
NLOC = 12800
TOPK = 8
QSCALE = 32.0

NBIG = 6              # bigpas of 2048 keys
REM = 512             # leftover keys
M1W = 6400


def build_dispatch_a(bq=B, nloc=NLOC, dbg=False):
    assert nloc == NBIG * 2048 + REM
    qtiles = bq // 128
    xbatches = qtiles // 4
    kbatches = nloc // 512          # 25
    ngroups = NBIG                  # groups of 4 batches; batch 24 = rem

    nc = bacc.Bacc("TRN2", target_bir_lowering=False, debug=dbg)
    x_d = nc.dram_tensor("x", [bq, D], F32, kind="ExternalInput").ap()
    k_d = nc.dram_tensor("keys", [nloc, D], F32, kind="ExternalInput").ap()
    out_d = nc.dram_tensor("cand", [bq, 8], F32, kind="ExternalOutput").ap()
    kinv_d = nc.dram_tensor("kinv", [nloc, 1], F32, kind="ExternalOutput").ap()

    with tile.TileContext(nc) as tc:
        with (
            tc.tile_pool(name="const", bufs=1) as constp,
            tc.tile_pool(name="ktfp", bufs=9) as ktfp,
            tc.tile_pool(name="kprep", bufs=4) as kprep,
            tc.tile_pool(name="xprep", bufs=2) as xprep,
            tc.tile_pool(name="big", bufs=1) as bigp,
            tc.tile_pool(name="s16", bufs=4) as s16p,
            tc.tile_pool(name="sc", bufs=2) as scpool,
            tc.tile_pool(name="ps", bufs=2, space="PSUM") as psp,
        ):
            identf = constp.tile([128, 128], F32)
            make_identity(nc, identf[:])
            identf8 = constp.tile([128, 128], FP8)
            make_identity(nc, identf8[:])
            eps = constp.tile([128, 1], F32)
            nc.gpsimd.memset(eps[:], 1e-30)

            kT = bigp.tile([128, 2, nloc], FP8)
            xT = bigp.tile([128, 2, bq], FP8)
            kinv_all = bigp.tile([128, kbatches * 4], F32)
            kinv32_all = bigp.tile([128, kbatches * 4], F32)

            m1ps = [bigp.tile([128, M1W], I32, tag=f"m1p{bi}", name=f"m1p{bi}")
                    for bi in range(2)]
            nc.gpsimd.iota(m1ps[0][:], pattern=[[1, M1W]], base=0,
                           channel_multiplier=0)
            # second iota buffer via sbuf->sbuf DMA copy
            nc.sync.dma_start(out=m1ps[1][:], in_=m1ps[0][:])

            def hi16(t, lo, hi):
                v = t[:].bitcast(I16).rearrange("p (n two) -> p n two", two=2)
                return v[:, lo:hi, 1].bitcast(F16)

            # ---------- x prep (4 batches of 512 queries) ----------
            def emit_xbatch(b):
                xt4 = xprep.tile([128, 4, D], F32, tag="xt4")
                nc.sync.dma_start(
                    out=xt4[:],
                    in_=x_d[b * 512:(b + 1) * 512, :].rearrange(
                        "(f p) d -> p f d", p=128))
                xsq = xprep.tile([128, 4, D], F32, tag="xsq")
                nc.scalar.activation(xsq[:], xt4[:], ACTF.Square)
                xn2 = xprep.tile([128, 4], F32, tag="xn2")
                nc.vector.tensor_reduce(xn2[:], xsq[:], axis=AX.X, op=ALU.add)
                xsrt = xprep.tile([128, 4], F32, tag="xsrt")
                nc.scalar.activation(xsrt[:], xn2[:], ACTF.Sqrt, bias=eps[:])
                xinv32 = xprep.tile([128, 4], F32, tag="xinv32")
                nc.vector.reciprocal(xinv32[:], xsrt[:])
                nc.vector.tensor_scalar(xinv32[:], xinv32[:], QSCALE, None,
                                        op0=ALU.mult)
                xq4 = xprep.tile([128, 4, D], FP8, tag="xq4")
                nc.gpsimd.tensor_tensor(
                    xq4[:], xt4[:],
                    xinv32[:, :, None].to_broadcast([128, 4, D]),
                    op=ALU.mult)
                pt = psp.tile([128, 2048], F32, tag="pa")
                for f in range(4):
                    for c in range(2):
                        nc.tensor.matmul(
                            pt[:, (f * 2 + c) * 128:(f * 2 + c) * 128 + 128],
                            lhsT=xq4[:, f, c * 128:(c + 1) * 128],
                            rhs=identf8[:], start=True, stop=True)
                # pt layout [f, c, q] -> xT[:, c, 512b + 128f + q]
                dst = xT[:, :, b * 512:(b + 1) * 512].rearrange(
                    "p c (f q) -> p f c q", q=128)
                src = pt[:, :1024].rearrange("p (f c q) -> p f c q", c=2, q=128)
                nc.vector.tensor_copy(dst, src)

            # ---------- key prep ----------
            def emit_kbatch(b):
                ktf = ktfp.tile([128, 4, D], F32, tag="ktf")
                nc.sync.dma_start(
                    out=ktf[:],
                    in_=k_d[b * 512:(b + 1) * 512, :].rearrange(
                        "(f p) d -> p f d", p=128))
                ksq = kprep.tile([128, 4, D], F32, tag="ksq")
                if b % 2 == 0:
                    nc.scalar.activation(ksq[:], ktf[:], ACTF.Square)
                else:
                    nc.gpsimd.tensor_tensor(ksq[:], ktf[:], ktf[:],
                                            op=ALU.mult)
                nc.vector.tensor_reduce(kinv_all[:, b * 4:(b + 1) * 4],
                                        ksq[:], axis=AX.X, op=ALU.add)
                return ktf

            def emit_kfinalize(lo, hi):
                # kinv_all[:, lo:hi] currently holds sum of squares
                w = hi - lo
                ksrt = kprep.tile([128, 20], F32, tag="ksrt")
                nc.scalar.activation(ksrt[:, :w], kinv_all[:, lo:hi],
                                     ACTF.Sqrt, bias=eps[:])
                nc.vector.reciprocal(kinv_all[:, lo:hi], ksrt[:, :w])
                nc.vector.tensor_scalar(kinv32_all[:, lo:hi],
                                        kinv_all[:, lo:hi], QSCALE, None,
                                        op0=ALU.mult)

            def emit_ktransform(b, ktf):
                diag4 = kprep.tile([128, 4, 128], F32, tag="diag4")
                nc.gpsimd.tensor_tensor(
                    diag4[:],
                    identf[:, None, :].to_broadcast([128, 4, 128]),
                    kinv32_all[:, b * 4:(b + 1) * 4, None].to_broadcast(
                        [128, 4, 128]),
                    op=ALU.mult)
                pt = psp.tile([128, 2048], F32, tag="pa")
                for f in range(4):
                    for c in range(2):
                        nc.tensor.matmul(
                            pt[:, c * 1024 + f * 128:c * 1024 + f * 128 + 128],
                            lhsT=ktf[:, f, c * 128:(c + 1) * 128],
                            rhs=diag4[:, f, :], start=True, stop=True)
                dst = kT[:, :, b * 512:(b + 1) * 512]
                src = pt[:].rearrange("p (c n) -> p c n", c=2)[:, :, :512]
                nc.scalar.activation(dst, src, ACTF.Copy)

            # ---------- main loop pieces ----------
            def emit_bigpa(qt, g):
                m1p = m1ps[qt % 2]
                lhsT = xT[:, :, qt * 128:(qt + 1) * 128]
                pa = psp.tile([128, 2048], F32, tag="pa")
                for i in range(4):
                    nc.tensor.matmul(
                        pa[:, i * 512:(i + 1) * 512], lhsT=lhsT,
                        rhs=kT[:, :, g * 2048 + i * 512:g * 2048 + (i + 1) * 512],
                        start=True, stop=True,
                        perf_mode=mybir.MatmulPerfMode.DoubleRow)
                s16 = s16p.tile([128, 4, 512], F16, tag="s16")
                nc.scalar.activation(
                    s16[:], pa[:].rearrange("p (f n) -> p f n", f=4), ACTF.Copy)
                # packed dest: slots [1024g, 1024g+1024) as [2, 512] strided
                v = m1p[:, :NBIG * 1024].bitcast(I16).rearrange(
                    "p (gg h n two) -> p gg h n two", two=2, n=512, h=2)
                dst = v[:, g, :, :, 1].bitcast(F16)       # [128, 2, 512]
                nc.vector.tensor_tensor(dst, s16[:, 0::2, :], s16[:, 1::2, :],
                                        op=ALU.max)

            def emit_rem(qt):
                # rem 512 keys: ACT -> 2 f16 planes, DVE TT -> packed slots
                m1p = m1ps[qt % 2]
                lhsT = xT[:, :, qt * 128:(qt + 1) * 128]
                pa = psp.tile([128, 2048], F32, tag="pa")
                nc.tensor.matmul(
                    pa[:, :512], lhsT=lhsT,
                    rhs=kT[:, :, NBIG * 2048:NBIG * 2048 + 512],
                    start=True, stop=True,
                    perf_mode=mybir.MatmulPerfMode.DoubleRow)
                s16 = s16p.tile([128, 4, 512], F16, tag="s16")
                nc.scalar.activation(
                    s16[:, :2, :256],
                    pa[:, :512].rearrange("p (h n) -> p h n", h=2), ACTF.Copy)
                v = m1p[:].bitcast(I16).rearrange(
                    "p (n two) -> p n two", two=2)
                dst = v[:, NBIG * 1024:M1W, 1].bitcast(F16)    # [128, 256]
                nc.vector.tensor_tensor(dst, s16[:, 0, :256], s16[:, 1, :256],
                                        op=ALU.max)

            def emit_tail(qt):
                # true top-8 over all 6400 packed slots (no fold collisions)
                m1p = m1ps[qt % 2]
                top = scpool.tile([128, 8], F32, tag="top")
                nc.vector.max(out=top[:], in_=m1p[:].bitcast(F32))
                nc.sync.dma_start(out=out_d[qt * 128:(qt + 1) * 128, :],
                                  in_=top[:])

            # ---------- emission (groups software-pipelined by one) ----------
            emit_xbatch(0)
            ktfs = {}
            for b in range(4):
                ktfs[b] = emit_kbatch(b)
            for G in range(ngroups):
                # issue next group's loads/norms BEFORE this group's barrier
                if G < ngroups - 1:
                    for b in range(4 * G + 4, 4 * G + 8):
                        ktfs[b] = emit_kbatch(b)
                else:
                    ktfs[24] = emit_kbatch(24)
                emit_kfinalize(16 * G, 16 * G + 16)
                for b in range(4 * G, 4 * G + 4):
                    emit_ktransform(b, ktfs.pop(b))
                if G < xbatches - 1:
                    emit_xbatch(G + 1)
                emit_bigpa(0, G)
                if G >= 1:
                    emit_bigpa(1, G - 1)
            emit_kfinalize(96, 100)
            emit_ktransform(24, ktfs.pop(24))
            nc.sync.dma_start(
                out=kinv_d[:].rearrange("(t p) o -> p (t o)", p=128),
                in_=kinv_all[:])
            emit_rem(0)
            emit_tail(0)
            for g in (NBIG - 1,):
                emit_bigpa(1, g)
            emit_rem(1)
            emit_tail(1)

            for qt in range(2, qtiles):
                for g in range(NBIG):
                    emit_bigpa(qt, g)
                emit_rem(qt)
                emit_tail(qt)

    nc.compile()
    return nc


# --------------------------------------------------------------------------
# Host orchestration
# --------------------------------------------------------------------------

import time

import numpy as np

from concourse.bass_utils import run_bass_kernel_spmd

N = 100000
NCORES = 8
NPAD = NLOC * NCORES

_CACHE = {}
TRACE = False
last_exec_ns = (None, None)


def _run(nc, in_maps, core_ids):
    if TRACE:
        return run_bass_kernel_spmd(nc, in_maps, core_ids, trace=True)
    return run_bass_kernel_spmd(nc, in_maps, core_ids)


def _get_programs():
    if "A" not in _CACHE:
        _CACHE["A"] = build_dispatch_a()
    if "B" not in _CACHE:
        _CACHE["B"] = build_dispatch_b(B // NCORES)
    return _CACHE["A"], _CACHE["B"]


def _pair_perm():
    # global slot g = c*M1W + s -> its two member rows in the padded array
    s = np.arange(M1W)
    full = s < NBIG * 1024
    r0 = np.where(full, 2 * s - (s & 511), 12288 + (s - NBIG * 1024))
    stride = np.where(full, 512, 256)
    rows = np.stack([r0, r0 + stride], axis=1)
    return (np.arange(NCORES)[:, None, None] * NLOC
            + rows[None, :, :]).reshape(-1)


def kernel(x, keys, values, top_k):
    assert int(top_k) == TOPK
    x = np.ascontiguousarray(np.asarray(x, dtype=np.float32))
    keys = np.asarray(keys, dtype=np.float32)
    values = np.asarray(values, dtype=np.float32)
    assert x.shape == (B, D) and keys.shape == (N, D)

    keys_pad = np.zeros((NPAD, D), dtype=np.float32)
    keys_pad[:N] = keys
    values_pad = np.zeros((NPAD, D), dtype=np.float32)
    values_pad[:N] = values

    nc_a, nc_b = _get_programs()
    core_ids = list(range(NCORES))

    in_maps_a = [
        {"x": x,
         "keys": np.ascontiguousarray(keys_pad[c * NLOC:(c + 1) * NLOC])}
        for c in range(NCORES)
    ]
    t0 = time.perf_counter()
    res_a = _run(nc_a, in_maps_a, core_ids)
    t1 = time.perf_counter()
    cand = np.concatenate([res_a.results[c]["cand"] for c in range(NCORES)],
                          axis=1)
    kinv = np.concatenate([res_a.results[c]["kinv"] for c in range(NCORES)],
                          axis=0)
    keys_aug = np.concatenate([keys_pad, kinv.reshape(NPAD, 1)], axis=1)
    keyspair = np.ascontiguousarray(
        keys_aug[_pair_perm()].reshape(NCORES * M1W, 2 * 257).astype(
            np.float32))

    bs = B // NCORES
    in_maps_b = [
        {
            "vals": np.ascontiguousarray(cand[c * bs:(c + 1) * bs]),
            "x": np.ascontiguousarray(x[c * bs:(c + 1) * bs]),
            "keyspair": keyspair,
            "values": values_pad,
        }
        for c in range(NCORES)
    ]
    t2 = time.perf_counter()
    res_b = _run(nc_b, in_maps_b, core_ids)
    t3 = time.perf_counter()
    out = np.concatenate([res_b.results[c]["out"] for c in range(NCORES)],
                         axis=0)
    kernel.last_walltimes = (t1 - t0, t3 - t2)
    if TRACE:
        global last_exec_ns
        last_exec_ns = (res_a.exec_time_ns, res_b.exec_time_ns)
    return out.astype(np.float32)
